# revision 9
# baseline (speedup 1.0000x reference)
"""Bass/Trainium2 kernel for nn_BerpXposMultiHeadedAttention (8-core SPMD).

Sharding: data-parallel over batch (4 batches x 2 cores) x tensor-parallel over
heads (4 heads per core).  Each core computes its 4 heads of flash-style xpos
attention for its batch plus the row-sharded partial out-projection; the host
sums the two partials per batch (the "all-reduce") and adds the output bias.

Device layout notes:
- All inputs arrive pre-transposed from the host ([embed, seq] with an appended
  ones row so q/k/v biases fold into the projection matmuls).
- Scores are computed transposed (S.T [s, t]); the softmax denominator comes
  from an appended ones-column in V; the final per-row normalization is a
  per-partition scalar multiply in the [t, d] layout.
- xpos rotation uses a head-dim deinterleave permutation folded into Wq/Wk on
  the host; the rotate-half partner is produced by a PE permutation matmul
  (DVE cannot read across partitions).
- Matmuls run in fp32r (QK^T, projections) and bf16 (P@V, out-projection) with
  fp32 PSUM accumulation.
- Emission interleaves projection blocks with flash strips so all five engines
  ramp within ~15us.
"""

import sys

sys.path.insert(0, "/opt/trn_rl_repo")

import contextlib

import numpy as np

import concourse.bacc as bacc
import concourse.bass as bass
import concourse.tile as tile
from concourse import mybir
from concourse.bass_utils import run_bass_kernel_spmd

# Problem constants (hardcoded per the task contract).
B = 4
L = 2048
EMBED = 512
HEADS = 8
HD = 64
SCALE_BASE = 512
NEG = -1e9
N_CORES = 8
HPC = 4           # heads per core
TB = 512          # t-block (strip) width
NT = L // 128     # 16 t-chunks
NS = L // 128     # 16 s-chunks
NSTRIP = L // TB  # 4 strips

F32 = mybir.dt.float32
F32R = mybir.dt.float32r
BF16 = mybir.dt.bfloat16

# Deinterleave permutation of a 64-wide head dim: evens then odds.
_PERM64 = np.concatenate([np.arange(0, HD, 2), np.arange(1, HD, 2)])


def _xpos_tables():
    """Host-side xpos cos/sin tables in the permuted [d, t] layout.

    Returns (cq, sq, ck, sk), each [128, L] float32 (two heads' worth of rows,
    identical per head).  The 1/sqrt(HD) score scale is folded into the q pair.
    """
    d = HD
    base = ((np.arange(0, d, 2, dtype=np.float32) + np.float32(0.4 * d))
            / np.float32(1.4 * d)).astype(np.float32)                    # [32]
    min_pos = -(L // 2)
    power = (np.arange(min_pos, L + min_pos, dtype=np.float32)
             / np.float32(SCALE_BASE))                                   # [L]
    scale = (base[None, :] ** power[:, None]).astype(np.float32)         # [L, 32]
    half = d // 2
    inv_freq = (1.0 / (10000.0 ** (np.arange(half, dtype=np.float32) / half))
                ).astype(np.float32)
    sinusoid = np.arange(L, dtype=np.float32)[:, None] * inv_freq[None, :]
    sin = np.sin(sinusoid).astype(np.float32)
    cos = np.cos(sinusoid).astype(np.float32)

    def pack(cs, ss, fold):
        cs = (cs * fold).astype(np.float32)
        ss = (ss * fold).astype(np.float32)
        # permuted layout: rows 0:32 <- even orig dims, rows 32:64 <- odd.
        cos_p = np.concatenate([cs.T, cs.T], axis=0)      # [64, L]
        sin_p = np.concatenate([-ss.T, ss.T], axis=0)     # [64, L]
        return (np.concatenate([cos_p, cos_p], axis=0).astype(np.float32),
                np.concatenate([sin_p, sin_p], axis=0).astype(np.float32))

    inv_scale = (1.0 / scale).astype(np.float32)
    cq, sq = pack(cos * scale, sin * scale, np.float32(HD ** -0.5))
    ck, sk = pack(cos * inv_scale, sin * inv_scale, np.float32(1.0))
    return cq, sq, ck, sk


def _build_program(causal: bool, use_mask: bool, reps: int = 1):
    nc = bacc.Bacc("TRN2", target_bir_lowering=False, debug=False,
                   num_devices=N_CORES)

    # ---- DRAM I/O -------------------------------------------------------
    xqT = nc.dram_tensor("xqT", [513, L], F32R, kind="ExternalInput")
    xkT = nc.dram_tensor("xkT", [513, L], F32R, kind="ExternalInput")
    xvT = nc.dram_tensor("xvT", [513, L], F32R, kind="ExternalInput")
    wqT = nc.dram_tensor("wqT", [513, 256], F32R, kind="ExternalInput")
    wkT = nc.dram_tensor("wkT", [513, 256], F32R, kind="ExternalInput")
    wvT = nc.dram_tensor("wvT", [513, 256], F32R, kind="ExternalInput")
    woT = nc.dram_tensor("woT", [256, EMBED], BF16, kind="ExternalInput")
    cqD = nc.dram_tensor("cq", [128, L], F32, kind="ExternalInput")
    sqD = nc.dram_tensor("sq", [128, L], F32, kind="ExternalInput")
    ckD = nc.dram_tensor("ck", [128, L], F32, kind="ExternalInput")
    skD = nc.dram_tensor("sk", [128, L], F32, kind="ExternalInput")
    triD = nc.dram_tensor("tri", [128, 128], F32, kind="ExternalInput")
    idD = nc.dram_tensor("ident", [128, 128], BF16, kind="ExternalInput")
    pswD = nc.dram_tensor("pswap", [128, 128], F32R, kind="ExternalInput")
    maskD = None
    if use_mask:
        maskD = nc.dram_tensor("maskT", [L, L], F32, kind="ExternalInput")
    outp = nc.dram_tensor("outp", [L, EMBED], F32, kind="ExternalOutput")

    xin = {"q": xqT, "k": xkT, "v": xvT}
    win = {"q": wqT, "k": wkT, "v": wvT}
    tabin = {"cq": cqD, "sq": sqD, "ck": ckD, "sk": skD}

    with tile.TileContext(nc) as tc:
        with contextlib.ExitStack() as ctx:
            consts = ctx.enter_context(tc.tile_pool(name="consts", bufs=1))
            xpool = ctx.enter_context(tc.tile_pool(name="xpool", bufs=22))
            wpool = ctx.enter_context(tc.tile_pool(name="wpool", bufs=1))
            qkpool = ctx.enter_context(tc.tile_pool(name="qkpool", bufs=1))
            tabpool = ctx.enter_context(tc.tile_pool(name="tabpool", bufs=6))
            vpool = ctx.enter_context(tc.tile_pool(name="vpool", bufs=NS))
            tmp = ctx.enter_context(tc.tile_pool(name="tmp", bufs=2))
            ptpool = ctx.enter_context(tc.tile_pool(name="ptpool", bufs=10))
            small = ctx.enter_context(tc.tile_pool(name="small", bufs=8))
            opool = ctx.enter_context(tc.tile_pool(name="opool", bufs=3))
            mpool = None
            if use_mask:
                mpool = ctx.enter_context(tc.tile_pool(name="mpool", bufs=NS + 2))
            ps_s = ctx.enter_context(tc.tile_pool(name="ps_s", bufs=2, space="PSUM"))
            ps_w = ctx.enter_context(tc.tile_pool(name="ps_w", bufs=1, space="PSUM"))
            ps_pv = ctx.enter_context(tc.tile_pool(name="ps_pv", bufs=2, space="PSUM"))
            ps_tr = ctx.enter_context(tc.tile_pool(name="ps_tr", bufs=1, space="PSUM"))

            def body():
                # ---- stage-0 constants (small) ----
                psw_sb = consts.tile([128, 128], F32R, tag="pswap")
                nc.sync.dma_start(psw_sb[:], pswD[:])
                ones_sb = consts.tile([1, L], F32R, tag="ones")
                nc.sync.dma_start(ones_sb[:], xqT[512:513, :])
                tri_sb = consts.tile([128, 128], F32, tag="tri")
                if causal:
                    nc.sync.dma_start(tri_sb[:], triD[:])

                w_sb = {}

                def load_w(nm):
                    chunks = []
                    for c in range(4):
                        t = wpool.tile([128, 256], F32R, tag=f"w{nm}{c}",
                                       name=f"w{nm}{c}")
                        nc.sync.dma_start(t[:], win[nm][c * 128:(c + 1) * 128, :])
                        chunks.append(t)
                    bt = wpool.tile([1, 256], F32R, tag=f"w{nm}b", name=f"w{nm}b")
                    nc.sync.dma_start(bt[:], win[nm][512:513, :])
                    w_sb[nm] = (chunks, bt)

                load_w("q")

                attnT = [consts.tile([128, L], BF16, tag=f"attnT{c}",
                                     name=f"attnT{c}") for c in range(2)]

                qTt = [[None] * NSTRIP for _ in range(2)]  # [e][tb]
                kTt = [[None] * NSTRIP for _ in range(2)]
                vaug = [None] * NS

                def load_x_tb(nm, tb):
                    """[128,512] pieces of x^T chunk c, cols of strip tb."""
                    pieces = []
                    for c in range(4):
                        t = xpool.tile([128, TB], F32R, tag="x",
                                       name=f"x{nm}{c}_{tb}")
                        nc.sync.dma_start(
                            t[:], xin[nm][c * 128:(c + 1) * 128,
                                          tb * TB:(tb + 1) * TB])
                        pieces.append(t)
                    return pieces

                def load_tab(nm, tb):
                    t = tabpool.tile([128, TB], F32, tag=nm, name=f"{nm}{tb}")
                    nc.sync.dma_start(
                        t[:], tabin[nm][:, tb * TB:(tb + 1) * TB])
                    return t

                def proj_qk(nm, tb, xs, ctab, stab, dst):
                    w_chunks, w_bias = w_sb[nm]
                    for e in range(2):
                        ps = ps_s.tile([128, TB], F32, tag="s",
                                       name=f"ps_{nm}{e}_{tb}")
                        for c in range(4):
                            nc.tensor.matmul(
                                ps[:], w_chunks[c][:, e * 128:(e + 1) * 128],
                                xs[c][:], start=(c == 0), stop=False)
                        nc.tensor.matmul(
                            ps[:], w_bias[:, e * 128:(e + 1) * 128],
                            ones_sb[:, tb * TB:(tb + 1) * TB],
                            start=False, stop=True)
                        raw = tmp.tile([128, TB], F32R, tag="raw",
                                       name=f"raw{nm}{e}{tb}")
                        nc.any.tensor_copy(raw[:], ps[:])
                        psw = ps_w.tile([128, TB], F32, tag="w",
                                        name=f"psw{nm}{e}{tb}")
                        nc.tensor.matmul(psw[:], psw_sb[:], raw[:],
                                         start=True, stop=True)
                        t1 = tmp.tile([128, TB], F32, tag="t1",
                                      name=f"t1{nm}{e}{tb}")
                        nc.vector.tensor_mul(t1[:], raw[:], ctab[:])
                        ot = qkpool.tile([128, TB], F32R, tag=f"{nm}T{e}_{tb}",
                                         name=f"{nm}T{e}_{tb}")
                        t2 = tmp.tile([128, TB], F32, tag="t2",
                                      name=f"t2{nm}{e}{tb}")
                        nc.vector.tensor_mul(t2[:], psw[:], stab[:])
                        nc.vector.tensor_add(ot[:], t1[:], t2[:])
                        dst[e][tb] = ot

                def proj_v(tb, xs):
                    w_chunks, w_bias = w_sb["v"]
                    for j in range(4):
                        si = tb * 4 + j
                        ps = ps_w.tile([128, 256], F32, tag="w",
                                       name=f"ps_v{si}")
                        for c in range(4):
                            nc.tensor.matmul(
                                ps[:], xs[c][:, j * 128:(j + 1) * 128],
                                w_chunks[c][:], start=(c == 0), stop=False)
                        nc.tensor.matmul(
                            ps[:], ones_sb[:, si * 128:(si + 1) * 128],
                            w_bias[:], start=False, stop=True)
                        va = vpool.tile([128, HPC * 65], BF16, tag="vaug",
                                        name=f"vaug{si}")
                        va3 = va[:].rearrange("p (h c) -> p h c", c=65)
                        nc.vector.tensor_copy(
                            va3[:, :, 0:64],
                            ps[:].rearrange("p (h d) -> p h d", d=64))
                        nc.vector.memset(va3[:, :, 64:65], 1.0)
                        vaug[si] = va

                # id/wo loads sit after the first strip's critical DMAs.
                def late_consts():
                    id_sb = consts.tile([128, 128], BF16, tag="ident")
                    nc.sync.dma_start(id_sb[:], idD[:])
                    wo_sb = []
                    for c in range(2):
                        t = consts.tile([128, EMBED], BF16, tag=f"wo{c}",
                                        name=f"wo{c}")
                        nc.sync.dma_start(t[:], woT[c * 128:(c + 1) * 128, :])
                        wo_sb.append(t)
                    return id_sb, wo_sb

                def flash_strip(T, id_sb):
                    nsig = 4 * T + 4 if causal else NS
                    mtiles = None
                    if use_mask:
                        mtiles = []
                        for si in range(nsig):
                            mt = mpool.tile([128, TB], F32, tag="mask",
                                            name=f"m{T}_{si}")
                            nc.sync.dma_start(
                                mt[:], maskD[si * 128:(si + 1) * 128,
                                             T * TB:(T + 1) * TB])
                            mtiles.append(mt)
                    for h in range(HPC):
                        ht, hr = h // 2, (h % 2) * 64
                        pts = []
                        for g in range((nsig + 1) // 2):
                            ps2 = ps_s.tile([128, 1024], F32, tag="s",
                                            name=f"S{T}h{h}g{g}")
                            for u in range(2):
                                sig = g * 2 + u
                                if sig >= nsig:
                                    continue
                                j = sig - 4 * T
                                coff = 0
                                if causal and j >= 0:
                                    ncols = max(TB - j * 128, 256)
                                    coff = TB - ncols
                                nc.tensor.matmul(
                                    ps2[:, u * TB + coff:(u + 1) * TB],
                                    kTt[ht][sig // 4][hr:hr + 64,
                                                      (sig % 4) * 128:
                                                      (sig % 4 + 1) * 128],
                                    qTt[ht][T][hr:hr + 64, coff:TB],
                                    start=True, stop=True)
                                if causal and j >= 0:
                                    sl = slice(u * TB + j * 128,
                                               u * TB + (j + 1) * 128)
                                    nc.vector.tensor_add(ps2[:, sl], ps2[:, sl],
                                                         tri_sb[:])
                                if use_mask:
                                    sl = slice(u * TB, (u + 1) * TB)
                                    nc.vector.tensor_add(ps2[:, sl], ps2[:, sl],
                                                         mtiles[sig][:])
                            pt = ptpool.tile([128, 1024], BF16, tag="pt",
                                             name=f"P{T}h{h}g{g}")
                            nc.scalar.activation(pt[:], ps2[:],
                                                 mybir.ActivationFunctionType.Exp)
                            pts.append(pt)
                        for jt in range(4):
                            tau = 4 * T + jt
                            cnt = tau + 1 if causal else NS
                            po = ps_pv.tile([128, 65], F32, tag="pv",
                                            name=f"pv{T}h{h}t{jt}")
                            for sig in range(cnt):
                                g, u = sig // 2, sig % 2
                                nc.tensor.matmul(
                                    po[:],
                                    pts[g][:, u * TB + jt * 128:
                                           u * TB + (jt + 1) * 128],
                                    vaug[sig][:, h * 65:(h + 1) * 65],
                                    start=(sig == 0), stop=(sig == cnt - 1))
                            rc = small.tile([128, 1], F32, tag="rc",
                                            name=f"rc{T}h{h}t{jt}")
                            nc.vector.reciprocal(rc[:], po[:, 64:65])
                            an = small.tile([128, 64], BF16, tag="an",
                                            name=f"an{T}h{h}t{jt}")
                            nc.vector.tensor_scalar(
                                an[:], po[:, 0:64], rc[:], None,
                                op0=mybir.AluOpType.mult)
                            pt_ps = ps_tr.tile([128, 128], BF16, tag="tr",
                                               name=f"tr{T}h{h}t{jt}")
                            nc.tensor.transpose(pt_ps[0:64, :], an[:], id_sb[:])
                            nc.vector.tensor_copy(
                                attnT[ht][hr:hr + 64, tau * 128:(tau + 1) * 128],
                                pt_ps[0:64, :])

                id_sb = wo_sb = None
                for tb in range(NSTRIP):
                    xs = load_x_tb("q", tb)
                    ct, st = load_tab("cq", tb), load_tab("sq", tb)
                    proj_qk("q", tb, xs, ct, st, qTt)
                    if tb == 0:
                        load_w("k")
                    xs = load_x_tb("k", tb)
                    ct, st = load_tab("ck", tb), load_tab("sk", tb)
                    proj_qk("k", tb, xs, ct, st, kTt)
                    if tb == 0:
                        load_w("v")
                    xs = load_x_tb("v", tb)
                    proj_v(tb, xs)
                    if tb == 0:
                        id_sb, wo_sb = late_consts()
                    flash_strip(tb, id_sb)

                # ---- out projection (partial; host adds pair + bias) ----
                for tau in range(NT):
                    ps = ps_s.tile([128, EMBED], F32, tag="s",
                                   name=f"ps_o{tau}")
                    for c in range(2):
                        nc.tensor.matmul(
                            ps[:], attnT[c][:, tau * 128:(tau + 1) * 128],
                            wo_sb[c][:], start=(c == 0), stop=(c == 1))
                    osb = opool.tile([128, EMBED], F32, tag="osb",
                                     name=f"osb{tau}")
                    nc.any.tensor_copy(osb[:], ps[:])
                    nc.sync.dma_start(outp[tau * 128:(tau + 1) * 128, :], osb[:])

            if reps > 1:
                with tc.For_i(0, reps, 1,
                              hint_engines=(mybir.EngineType.PE,
                                            mybir.EngineType.Activation,
                                            mybir.EngineType.DVE,
                                            mybir.EngineType.SP,
                                            mybir.EngineType.Pool)):
                    body()
            else:
                body()

    nc.compile()
    return nc


_PROGRAM_CACHE = {}


def get_program(causal: bool, use_mask: bool, reps: int = 1):
    key = (causal, use_mask, reps)
    if key not in _PROGRAM_CACHE:
        _PROGRAM_CACHE[key] = _build_program(causal, use_mask, reps)
    return _PROGRAM_CACHE[key]


def _prep_in_maps(query, key, value, key_padding_mask, attn_mask,
                  Wq, bq, Wk, bk, Wv, bv, Wo, bo, use_mask):
    """Build the 8 per-core input dicts."""
    import ml_dtypes
    cq, sq, ck, sk = _xpos_tables()
    tri = np.where(np.arange(128)[None, :] >= np.arange(128)[:, None],
                   np.float32(0.0), np.float32(NEG)).astype(np.float32)
    ident = np.eye(128, dtype=np.float32).astype(ml_dtypes.bfloat16)
    psw = np.zeros((128, 128), np.float32)
    m_idx = np.arange(128)
    psw[m_idx ^ 32, m_idx] = 1.0  # lhsT[k, m] = 1 iff k == m^32

    # per-batch transposed inputs with ones row
    def aug_x(x):
        a = np.empty((513, L), np.float32)
        a[0:512] = np.ascontiguousarray(x.T)
        a[512] = 1.0
        return a

    xqTs = [aug_x(np.asarray(query[b], np.float32)) for b in range(B)]
    xkTs = [aug_x(np.asarray(key[b], np.float32)) for b in range(B)]
    xvTs = [aug_x(np.asarray(value[b], np.float32)) for b in range(B)]

    masks = None
    if use_mask:
        am = np.asarray(attn_mask, np.float32)
        kp = np.asarray(key_padding_mask)
        masks = []
        for b in range(B):
            m = am.copy()
            if kp[b].any():
                m = m + np.where(kp[b], np.float32(-1e30),
                                 np.float32(0.0))[None, :]
            masks.append(np.ascontiguousarray(m.T.astype(np.float32)))

    Wq = np.asarray(Wq, np.float32); bq = np.asarray(bq, np.float32)
    Wk = np.asarray(Wk, np.float32); bk = np.asarray(bk, np.float32)
    Wv = np.asarray(Wv, np.float32); bv = np.asarray(bv, np.float32)
    Wo = np.asarray(Wo, np.float32)

    in_maps = []
    for core in range(N_CORES):
        b, hg = core // 2, core % 2
        hs = hg * HPC
        idx_p = np.concatenate(
            [hs * HD + hl * HD + _PERM64 for hl in range(HPC)])
        idx_v = hs * HD + np.arange(HPC * HD)

        def aug_w(W, bias, idx):
            a = np.empty((513, 256), np.float32)
            a[0:512] = np.ascontiguousarray(W[idx, :].T)
            a[512] = bias[idx]
            return a

        m = {
            "xqT": xqTs[b], "xkT": xkTs[b], "xvT": xvTs[b],
            "wqT": aug_w(Wq, bq, idx_p),
            "wkT": aug_w(Wk, bk, idx_p),
            "wvT": aug_w(Wv, bv, idx_v),
            "woT": np.ascontiguousarray(Wo[:, idx_v].T).astype(ml_dtypes.bfloat16),
            "cq": cq, "sq": sq, "ck": ck, "sk": sk,
            "tri": tri, "ident": ident, "pswap": psw,
        }
        if use_mask:
            m["maskT"] = masks[b]
        in_maps.append(m)
    return in_maps


def classify_mask(attn_mask, key_padding_mask):
    am = np.asarray(attn_mask, np.float32)
    kp = np.asarray(key_padding_mask)
    if not kp.any():
        causal = np.where(
            np.tril(np.ones((L, L), bool)), np.float32(0.0),
            np.float32(NEG)).astype(np.float32)
        if np.array_equal(am, causal):
            return True, False
        if not am.any():
            return False, False
    return False, True


def kernel(query, key, value, key_padding_mask, attn_mask,
           Wq, bq, Wk, bk, Wv, bv, Wo, bo):
    causal, use_mask = classify_mask(attn_mask, key_padding_mask)
    nc = get_program(causal, use_mask, reps=1)
    in_maps = _prep_in_maps(query, key, value, key_padding_mask, attn_mask,
                            Wq, bq, Wk, bk, Wv, bv, Wo, bo, use_mask)
    res = run_bass_kernel_spmd(nc, in_maps, list(range(N_CORES)))
    bo = np.asarray(bo, np.float32)
    out = np.empty((B, L, EMBED), np.float32)
    for b in range(B):
        out[b] = (res.results[2 * b]["outp"]
                  + res.results[2 * b + 1]["outp"] + bo[None, :])
    return out
